# revision 13
# baseline (speedup 1.0000x reference)
"""Trainium2 Bass kernel for nn_LoRALinear4bit.

Computes  out = x @ dequant_nf4(q_idx, absmax).T + (x @ A) @ B * 2.0
with x [4,2048,4096] f32, q_idx [4096,4096] int32 (NF4 codes),
absmax [4096,64] f32 (per-64-block scales), A [4096,16], B [16,4096].

Strategy (column / tensor parallel over 8 NeuronCores):
  * shard out_features OUT=4096 into 8 x 512; replicate x, A.
  * all matmuls in bf16 (tolerance is 2e-2; this path measures ~1e-2):
    halves the dominant x DMA traffic (134 MB -> 67 MB per core) that made
    the fp32 version DMA-bound.
  * dequant: host sends u = fp16((2q-15)/15); device evaluates a degree-7
    least-squares polynomial through those 16 (fp16-rounded) nodes with an
    fp16 Horner chain on DVE, times the expanded absmax (bf16).
    LoRA fold: 2*(A@B) via a tiny PE matmul into PSUM; the DVE adds it
    straight from PSUM into the bf16 weff tile (no intermediate copy, so
    the dequant pipeline never waits on the ACT stream, which is busy with
    PE-gated partial flushes).
  * phase B: out_shard = x @ W_eff accumulated over 32 k-tiles in PSUM.
    Split-group machinery overlaps it with dequant: the first NSPLIT token
    groups run k<16 in chunks (flushed to bf16 partials, re-injected via
    identity matmul), emitted interleaved with phase A in weff-availability
    order so the PE stream never head-of-line blocks on unproduced tiles.
  * ~24 throwaway warm-up matmuls at t=0 lift the PE HAM clock gate to
    K=8/8 before the first real matmul lands.
  * x is DMA'd from a host-side [group][feat%128][feat//128][token] layout
    so every transfer is 2-8 KB contiguous per partition line.

Host-side work is layout/dtype only: transposes, shard slicing, absmax
block expansion, and f32->bf16/fp16 casts.
"""

import numpy as np
import ml_dtypes

# problem shape (hardcoded per contract: kernel.py must be self-contained)
B_, S_, IN, OUT = 4, 2048, 4096, 4096
TOK = B_ * S_            # 8192 tokens
NCORES = 8
OSH = OUT // NCORES      # 512 out-features per core
R = 16                   # LoRA rank
SCALING = 2.0            # alpha/r = 32/16
QBLOCK = 64              # bnb absmax blocksize

KT = IN // 128           # 32 K tiles
TG = 512                 # token group
NG = TOK // TG           # 16 token groups
MPG = TG // 128          # 4 m-tiles per group
KH = KT // 2             # 16: B1/B2 split point
NSPLIT = 14              # groups whose contraction is split k<KH | k>=KH
DEG = 7                  # dequant polynomial degree
XSLAB = 16               # k-tiles per x DMA slab
NWARM = 32               # PE warm-up matmuls

# bitsandbytes NF4 codebook
NF4 = np.array([
    -1.0, -0.6961928009986877, -0.5250730514526367, -0.39491748809814453,
    -0.28444138169288635, -0.18477343022823334, -0.09105003625154495, 0.0,
    0.07958029955625534, 0.16093020141124725, 0.24611230194568634,
    0.33791524171829224, 0.44070982933044434, 0.5626170039176941,
    0.6989699602127075, 1.0], dtype=np.float64)


def _poly_coeffs():
    """Degree-DEG least-squares fit of the NF4 codebook at the fp16-rounded
    nodes u = fp16((2q-15)/15) (monomial basis, increasing order).
    rms error ~0.8% of the codebook rms — inside the 2e-2 gate."""
    q = np.arange(16, dtype=np.float64)
    u = ((2.0 * q - 15.0) / 15.0).astype(np.float16).astype(np.float64)
    V = np.vander(u, DEG + 1, increasing=True)
    c, *_ = np.linalg.lstsq(V, NF4, rcond=None)
    return c


_CACHE = {}


def _build():
    """Build + compile the per-core Bass program (identical on all cores)."""
    if "nc" in _CACHE:
        return _CACHE["nc"]

    import concourse.bacc as bacc
    import concourse.tile as tile
    from concourse import mybir
    from concourse.bass import ts, ds

    f32 = mybir.dt.float32
    f16 = mybir.dt.float16
    bf16 = mybir.dt.bfloat16
    Alu = mybir.AluOpType

    c = _poly_coeffs()

    nc = bacc.Bacc("TRN2", target_bir_lowering=False, debug=False)

    # x in grouped layout: row g*128+p holds [kt, t] flattened
    xt = nc.dram_tensor("xt", [NG * 128, KT * TG], bf16,
                        kind="ExternalInput").ap()
    ident = nc.dram_tensor("ident", [128, 128], bf16, kind="ExternalInput").ap()
    wrm = nc.dram_tensor("wrm", [128, OSH], bf16, kind="ExternalInput").ap()
    ut = nc.dram_tensor("ut", [IN, OSH], f16, kind="ExternalInput").ap()
    scl = nc.dram_tensor("scl", [IN, OSH], bf16, kind="ExternalInput").ap()
    at = nc.dram_tensor("at", [R, IN], f32, kind="ExternalInput").ap()
    bsh = nc.dram_tensor("bsh", [R, OSH], f32, kind="ExternalInput").ap()
    out = nc.dram_tensor("out", [TOK, OSH], f32, kind="ExternalOutput").ap()

    with tile.TileContext(nc) as tc:
        with (
            tc.tile_pool(name="weff", bufs=1) as weff_pool,
            tc.tile_pool(name="deq", bufs=5) as deq_pool,
            tc.tile_pool(name="part", bufs=1) as part_pool,
            tc.tile_pool(name="xin", bufs=4) as x_pool,
            tc.tile_pool(name="oup", bufs=8) as o_pool,
            tc.tile_pool(name="wadd_ps", bufs=2, space="PSUM") as wadd_pool,
            tc.tile_pool(name="mm_ps", bufs=6, space="PSUM") as mm_pool,
            tc.tile_pool(name="const", bufs=1) as const_pool,
        ):
            # resident constants
            b_sb = const_pool.tile([R, OSH], f32, tag="b_sb", name="b_sb")
            nc.gpsimd.dma_start(out=b_sb[:], in_=bsh[:])
            id_sb = const_pool.tile([128, 128], bf16, tag="id_sb", name="id_sb")
            nc.gpsimd.dma_start(out=id_sb[:], in_=ident[:])
            wrm_sb = const_pool.tile([128, OSH], bf16, tag="wrm_sb",
                                     name="wrm_sb")
            nc.sync.dma_start(out=wrm_sb[:], in_=wrm[:])

            # PE warm-up: throwaway matmuls lift the HAM clock gate to 8/8
            # before the first real matmul (~8us in).  Output is discarded.
            # Uses an mm_pool bank so the LoRA wadd matmuls are not delayed.
            wu = mm_pool.tile([128, OSH], f32, tag="mmps", name="warm")
            for _ in range(NWARM):
                nc.tensor.matmul(wu[:], id_sb[:], wrm_sb[:], start=True,
                                 stop=True)

            weff = []
            for j in range(KT):
                weff.append(weff_pool.tile([128, OSH], bf16, tag=f"weff{j}",
                                           name=f"weff{j}"))

            # ---- Phase A tile: W_eff[j] = poly(u)*scale + 2*(A@B)
            # Inputs + the LoRA matmul are emitted one tile AHEAD of the
            # chain: AB(j) must land in the PE stream BEFORE the big B1
            # chunk waves, or the DVE weff-add (and with it the whole
            # dequant pipeline) stalls behind ~100us of queued matmuls.
            deq_in = {}
            wadds = {}

            def emit_inputs(j):
                # HWDGE (sync) for the first tiles so the pipeline head is
                # short; SWDGE (gpsimd) for the rest so the deq feed never
                # queues behind PE-gated work on the sync/scalar rings.
                dma = nc.sync.dma_start if j < 2 else nc.gpsimd.dma_start
                utl = deq_pool.tile([128, OSH], f16, tag="utl", name="utl")
                dma(out=utl[:], in_=ut[ts(j, 128), :])
                sctl = deq_pool.tile([128, OSH], bf16, tag="sctl", name="sctl")
                dma(out=sctl[:], in_=scl[ts(j, 128), :])
                atl = deq_pool.tile([R, 128], f32, tag="atl", name="atl")
                dma(out=atl[:], in_=at[:, ts(j, 128)])
                deq_in[j] = (utl, sctl)

                # LoRA fold: wadd = (A @ 2B)[j]  (psum, exact fp32)
                wadd = wadd_pool.tile([128, OSH], f32, tag="wadd", name="wadd")
                nc.tensor.matmul(wadd[:], atl[:], b_sb[:], start=True,
                                 stop=True)
                wadds[j] = wadd

            def emit_chain(j):
                eng = nc.vector
                utl, sctl = deq_in.pop(j)
                # Horner in fp16:
                #   acc = c[DEG]*u;  acc = (acc + c[k])*u  for k=DEG-1..1
                acc = deq_pool.tile([128, OSH], f16, tag="acc", name="acc")
                eng.tensor_scalar_mul(acc[:], utl[:], float(c[DEG]))
                for kk in range(DEG - 1, 0, -1):
                    eng.scalar_tensor_tensor(
                        acc[:], acc[:], float(c[kk]), utl[:],
                        Alu.add, Alu.mult)
                # tmp = (acc + c0) * absmax_expanded
                tmp = deq_pool.tile([128, OSH], bf16, tag="tmp", name="tmp")
                eng.scalar_tensor_tensor(
                    tmp[:], acc[:], float(c[0]), sctl[:], Alu.add, Alu.mult)
                # weff = wadd(PSUM) + tmp on DVE (frees the bank)
                nc.vector.tensor_add(weff[j][:], wadds.pop(j)[:], tmp[:])

            # ---- Phase B helpers
            # m-outer matmul order: each psum's first write (re-inject or
            # first MM) sits one m-segment after the previous, so with a
            # 6-deep psum pool the flush of the previous chunk overlaps the
            # start of the next without stalling the PE.
            def mm_span(g, k0, k1, psums, reinj, stop_last):
                slabs = []
                k = k0
                while k < k1:
                    ke = min(k + XSLAB, k1)
                    xg = x_pool.tile([128, XSLAB * TG], bf16, tag="xg",
                                     name="xg")
                    nc.sync.dma_start(
                        out=xg[:, ds(0, (ke - k) * TG)],
                        in_=xt[ds(g * 128, 128), ds(k * TG, (ke - k) * TG)])
                    slabs.append((k, ke, xg))
                    k = ke
                for m in range(MPG):
                    if reinj is not None:
                        nc.tensor.matmul(psums[m][:], id_sb[:],
                                         reinj[m][:], start=True, stop=False)
                    for kb, ke, xg in slabs:
                        for s in range(ke - kb):
                            kk = kb + s
                            nc.tensor.matmul(
                                psums[m][:],
                                xg[:, ds(s * TG + m * 128, 128)],
                                weff[kk][:],
                                start=(reinj is None) and (kk == k0),
                                stop=stop_last and (kk == k1 - 1))

            partials = {}

            def emit_chunk(g, ci, k0, k1):
                psums = [mm_pool.tile([128, OSH], f32, tag="mmps",
                                      name="mmps") for _ in range(MPG)]
                reinj = [partials[(g, m)] for m in range(MPG)] if ci > 0 \
                    else None
                mm_span(g, k0, k1, psums, reinj, True)
                for m in range(MPG):
                    if ci == 0:
                        partials[(g, m)] = part_pool.tile(
                            [128, OSH], bf16, tag=f"part{g}_{m}",
                            name=f"part{g}_{m}")
                    nc.scalar.copy(partials[(g, m)][:], psums[m][:])

            def emit_b2(g):
                psums = [mm_pool.tile([128, OSH], f32, tag="mmps",
                                      name="mmps") for _ in range(MPG)]
                mm_span(g, KH, KT, psums, None, True)
                for m in range(MPG):
                    ot = o_pool.tile([128, OSH], f32, tag="ot", name="ot")
                    nc.vector.tensor_add(ot[:], psums[m][:],
                                         partials[(g, m)][:])
                    nc.scalar.dma_start(
                        out=out[ds(g * TG + m * 128, 128), :], in_=ot[:])

            def emit_b3(g):
                psums = [mm_pool.tile([128, OSH], f32, tag="mmps",
                                      name="mmps") for _ in range(MPG)]
                mm_span(g, 0, KT, psums, None, True)
                for m in range(MPG):
                    ot = o_pool.tile([128, OSH], f32, tag="ot", name="ot")
                    nc.scalar.copy(ot[:], psums[m][:])
                    nc.scalar.dma_start(
                        out=out[ds(g * TG + m * 128, 128), :], in_=ot[:])

            # B1 chunk table: fine chunks for the first groups so the PE has
            # work as soon as weff[0] lands; coarse (one flush) for the rest.
            bounds_for = {0: [0, 1, 2, 4, 8, 12, KH],
                          1: [0, 2, 4, 8, 12, KH],
                          2: [0, 4, 8, 12, KH]}
            by_k1 = {}
            for g in range(NSPLIT):
                bounds = bounds_for.get(g, [0, 8, KH])
                for ci in range(len(bounds) - 1):
                    by_k1.setdefault(bounds[ci + 1], []).append(
                        (g, ci, bounds[ci]))

            # Interleave phase A with the B1 chunks that each new weff tile
            # unlocks, so both the dequant chains and the PE stream flow in
            # weff-availability order.  Inputs/AB lead the chain by one tile.
            emit_inputs(0)
            for j in range(KT):
                if j + 1 < KT:
                    emit_inputs(j + 1)
                emit_chain(j)
                for g, ci, k0 in by_k1.get(j + 1, []):
                    emit_chunk(g, ci, k0, j + 1)

            # Tail: second halves of split groups + the unsplit groups.
            b2s = list(range(NSPLIT))
            b3s = list(range(NSPLIT, NG))
            order = []
            while b2s or b3s:
                if b2s:
                    order.append(("b2", b2s.pop(0)))
                if b3s:
                    order.append(("b3", b3s.pop(0)))
            for kind, g in order:
                (emit_b2 if kind == "b2" else emit_b3)(g)

    nc.compile()
    _CACHE["nc"] = nc
    return nc


def _prepare_in_maps(x, q_idx, absmax, lora_A, lora_B):
    x = np.asarray(x, dtype=np.float32)
    q_idx = np.asarray(q_idx, dtype=np.int32)
    absmax = np.asarray(absmax, dtype=np.float32)
    lora_A = np.asarray(lora_A, dtype=np.float32)
    lora_B = np.asarray(lora_B, dtype=np.float32)

    bf = ml_dtypes.bfloat16
    # grouped x layout: [g, t, kt, p] -> [g, p, kt, t] -> row g*128+p
    x2 = np.ascontiguousarray(
        x.reshape(TOK, IN).reshape(NG, TG, KT, 128).transpose(0, 3, 2, 1)
    ).reshape(NG * 128, KT * TG).astype(bf)
    # u = fp16((2q-15)/15), transposed  [IN, OUT]
    u_full = ((2.0 * q_idx.T.astype(np.float32) - 15.0) / 15.0
              ).astype(np.float16)
    at = np.ascontiguousarray(lora_A.T)                      # [R, IN]
    ident = np.eye(128, dtype=np.float32).astype(bf)
    wrm = np.zeros((128, OSH), dtype=bf)

    in_maps = []
    for cid in range(NCORES):
        sl = slice(cid * OSH, (cid + 1) * OSH)
        scale = np.repeat(np.ascontiguousarray(absmax[sl].T), QBLOCK,
                          axis=0).astype(bf)                 # [IN, OSH]
        in_maps.append({
            "xt": x2,
            "ident": ident,
            "wrm": wrm,
            "ut": np.ascontiguousarray(u_full[:, sl]),
            "scl": scale,
            "at": at,
            "bsh": np.ascontiguousarray(SCALING * lora_B[:, sl]),
        })
    return in_maps


def _gather(results):
    shards = [results[cid]["out"] for cid in range(NCORES)]
    full = np.concatenate(shards, axis=1)                    # [TOK, OUT]
    return full.reshape(B_, S_, OUT)


def kernel(x, q_idx, absmax, lora_A, lora_B):
    from concourse.bass_utils import run_bass_kernel_spmd

    nc = _build()
    in_maps = _prepare_in_maps(x, q_idx, absmax, lora_A, lora_B)
    res = run_bass_kernel_spmd(nc, in_maps, list(range(NCORES)))
    return _gather(res.results)


# revision 20
# speedup vs baseline: 1.0536x; 1.0536x over previous
"""Trainium2 Bass kernel for nn_LoRALinear4bit.

Computes  out = x @ dequant_nf4(q_idx, absmax).T + (x @ A) @ B * 2.0
with x [4,2048,4096] f32, q_idx [4096,4096] int32 (NF4 codes),
absmax [4096,64] f32 (per-64-block scales), A [4096,16], B [16,4096].

Strategy (column / tensor parallel over 8 NeuronCores):
  * shard out_features OUT=4096 into 8 x 512; replicate x, A.
  * all matmuls in bf16 (tolerance is 2e-2; this path measures ~1e-2):
    halves the dominant x DMA traffic (134 MB -> 67 MB per core) that made
    the fp32 version DMA-bound.
  * dequant: host sends u = fp16((2q-15)/15); device evaluates a degree-7
    least-squares polynomial through those 16 (fp16-rounded) nodes with an
    fp16 Horner chain on DVE, times the expanded absmax (bf16).
    LoRA fold: 2*(A@B) via a tiny PE matmul into PSUM; the DVE adds it
    straight from PSUM into the bf16 weff tile (no intermediate copy, so
    the dequant pipeline never waits on the ACT stream, which is busy with
    PE-gated partial flushes).
  * phase B: out_shard = x @ W_eff accumulated over 32 k-tiles in PSUM.
    Split-group machinery overlaps it with dequant: the first NSPLIT token
    groups run k<16 in chunks (flushed to bf16 partials, re-injected via
    identity matmul), emitted interleaved with phase A in weff-availability
    order so the PE stream never head-of-line blocks on unproduced tiles.
  * ~24 throwaway warm-up matmuls at t=0 lift the PE HAM clock gate to
    K=8/8 before the first real matmul lands.
  * x is DMA'd from a host-side [group][feat%128][feat//128][token] layout
    so every transfer is 2-8 KB contiguous per partition line.

Host-side work is layout/dtype only: transposes, shard slicing, absmax
block expansion, and f32->bf16/fp16 casts.
"""

import numpy as np
import ml_dtypes

# problem shape (hardcoded per contract: kernel.py must be self-contained)
B_, S_, IN, OUT = 4, 2048, 4096, 4096
TOK = B_ * S_            # 8192 tokens
NCORES = 8
OSH = OUT // NCORES      # 512 out-features per core
R = 16                   # LoRA rank
SCALING = 2.0            # alpha/r = 32/16
QBLOCK = 64              # bnb absmax blocksize

KT = IN // 128           # 32 K tiles
TG = 512                 # token group
NG = TOK // TG           # 16 token groups
MPG = TG // 128          # 4 m-tiles per group
KH = KT // 2             # 16: B1/B2 split point
NSPLIT = 14              # groups whose contraction is split k<KH | k>=KH
DEG = 7                  # dequant polynomial degree
XSLAB = 8                # k-tiles per x DMA slab
NWARM = 12               # PE warm-up matmuls

# bitsandbytes NF4 codebook
NF4 = np.array([
    -1.0, -0.6961928009986877, -0.5250730514526367, -0.39491748809814453,
    -0.28444138169288635, -0.18477343022823334, -0.09105003625154495, 0.0,
    0.07958029955625534, 0.16093020141124725, 0.24611230194568634,
    0.33791524171829224, 0.44070982933044434, 0.5626170039176941,
    0.6989699602127075, 1.0], dtype=np.float64)


def _poly_coeffs():
    """Degree-DEG least-squares fit of the NF4 codebook at the fp16-rounded
    nodes u = fp16((2q-15)/15) (monomial basis, increasing order).
    rms error ~0.8% of the codebook rms — inside the 2e-2 gate."""
    q = np.arange(16, dtype=np.float64)
    u = ((2.0 * q - 15.0) / 15.0).astype(np.float16).astype(np.float64)
    V = np.vander(u, DEG + 1, increasing=True)
    c, *_ = np.linalg.lstsq(V, NF4, rcond=None)
    return c


_CACHE = {}


def _build():
    """Build + compile the per-core Bass program (identical on all cores)."""
    if "nc" in _CACHE:
        return _CACHE["nc"]

    import concourse.bacc as bacc
    import concourse.tile as tile
    from concourse import mybir
    from concourse.bass import ts, ds

    f32 = mybir.dt.float32
    f16 = mybir.dt.float16
    bf16 = mybir.dt.bfloat16
    Alu = mybir.AluOpType

    c = _poly_coeffs()

    nc = bacc.Bacc("TRN2", target_bir_lowering=False, debug=False)

    # x in grouped layout: row g*128+p holds [kt, t] flattened
    xt = nc.dram_tensor("xt", [NG * 128, KT * TG], bf16,
                        kind="ExternalInput").ap()
    ident = nc.dram_tensor("ident", [128, 128], bf16, kind="ExternalInput").ap()
    wrm = nc.dram_tensor("wrm", [128, OSH], bf16, kind="ExternalInput").ap()
    ut = nc.dram_tensor("ut", [IN, OSH], f16, kind="ExternalInput").ap()
    scl = nc.dram_tensor("scl", [IN, OSH], bf16, kind="ExternalInput").ap()
    at = nc.dram_tensor("at", [R, IN], bf16, kind="ExternalInput").ap()
    bsh = nc.dram_tensor("bsh", [R, OSH], bf16, kind="ExternalInput").ap()
    out = nc.dram_tensor("out", [TOK, OSH], f32, kind="ExternalOutput").ap()

    with tile.TileContext(nc) as tc:
        with (
            tc.tile_pool(name="weff", bufs=1) as weff_pool,
            tc.tile_pool(name="deq", bufs=5) as deq_pool,
            tc.tile_pool(name="part", bufs=1) as part_pool,
            tc.tile_pool(name="xin", bufs=4) as x_pool,
            tc.tile_pool(name="oup", bufs=8) as o_pool,
            tc.tile_pool(name="wadd_ps", bufs=2, space="PSUM") as wadd_pool,
            tc.tile_pool(name="mm_ps", bufs=6, space="PSUM") as mm_pool,
            tc.tile_pool(name="const", bufs=1) as const_pool,
        ):
            # resident constants
            b_sb = const_pool.tile([R, OSH], bf16, tag="b_sb", name="b_sb")
            nc.gpsimd.dma_start(out=b_sb[:], in_=bsh[:])
            id_sb = const_pool.tile([128, 128], bf16, tag="id_sb", name="id_sb")
            nc.gpsimd.dma_start(out=id_sb[:], in_=ident[:])
            at_sb = const_pool.tile([R, IN], bf16, tag="at_sb", name="at_sb")
            nc.gpsimd.dma_start(out=at_sb[:], in_=at[:])
            wrm_sb = const_pool.tile([128, OSH], bf16, tag="wrm_sb",
                                     name="wrm_sb")
            nc.scalar.dma_start(out=wrm_sb[:], in_=wrm[:])

            # PE warm-up: throwaway matmuls lift the HAM clock gate to 8/8
            # before the first real matmul (~8us in).  Output is discarded.
            # Uses an mm_pool bank so the LoRA wadd matmuls are not delayed.
            wu = mm_pool.tile([128, OSH], f32, tag="mmps", name="warm")
            for _ in range(NWARM):
                nc.tensor.matmul(wu[:], id_sb[:], wrm_sb[:], start=True,
                                 stop=True)

            weff = []
            for j in range(KT):
                weff.append(weff_pool.tile([128, OSH], bf16, tag=f"weff{j}",
                                           name=f"weff{j}"))

            # ---- LoRA fold: all 32 wadd2[j] = (A @ 2B)[j] products are
            # computed up-front (spread over the first few loop iterations)
            # and ACT-copied out of PSUM into resident bf16 tiles BEFORE any
            # PE-gated partial flush enters the ACT stream.  The dequant
            # chain then depends on nothing that waits on the B1 waves —
            # otherwise the in-order DVE stream stalls ~100us at each
            # weff-add whose LoRA matmul is queued behind a chunk wave.
            wadd2 = {}

            def emit_ab(j):
                wadd = wadd_pool.tile([128, OSH], f32, tag="wadd", name="wadd")
                nc.tensor.matmul(wadd[:], at_sb[:, ts(j, 128)], b_sb[:],
                                 start=True, stop=True)
                w2 = part_pool.tile([128, OSH], bf16, tag=f"w2_{j}",
                                    name=f"w2_{j}")
                nc.scalar.copy(w2[:], wadd[:])
                wadd2[j] = w2

            deq_in = {}

            def emit_inputs(j):
                # HWDGE for the first tiles so the pipeline head is short;
                # SWDGE (gpsimd) for the rest so the deq feed never queues
                # behind PE-gated work on the sync/scalar rings.
                if j < 2:
                    utl = deq_pool.tile([128, OSH], f16, tag="utl", name="utl")
                    nc.sync.dma_start(out=utl[:], in_=ut[ts(j, 128), :])
                    sctl = deq_pool.tile([128, OSH], bf16, tag="sctl",
                                         name="sctl")
                    nc.scalar.dma_start(out=sctl[:], in_=scl[ts(j, 128), :])
                else:
                    utl = deq_pool.tile([128, OSH], f16, tag="utl", name="utl")
                    nc.gpsimd.dma_start(out=utl[:], in_=ut[ts(j, 128), :])
                    sctl = deq_pool.tile([128, OSH], bf16, tag="sctl",
                                         name="sctl")
                    nc.gpsimd.dma_start(out=sctl[:], in_=scl[ts(j, 128), :])
                deq_in[j] = (utl, sctl)

            def emit_chain(j):
                eng = nc.vector
                utl, sctl = deq_in.pop(j)
                # Horner in fp16:
                #   acc = c[DEG]*u;  acc = (acc + c[k])*u  for k=DEG-1..1
                acc = deq_pool.tile([128, OSH], f16, tag="acc", name="acc")
                eng.tensor_scalar_mul(acc[:], utl[:], float(c[DEG]))
                for kk in range(DEG - 1, 0, -1):
                    eng.scalar_tensor_tensor(
                        acc[:], acc[:], float(c[kk]), utl[:],
                        Alu.add, Alu.mult)
                # tmp = (acc + c0) * absmax_expanded
                tmp = deq_pool.tile([128, OSH], bf16, tag="tmp", name="tmp")
                eng.scalar_tensor_tensor(
                    tmp[:], acc[:], float(c[0]), sctl[:], Alu.add, Alu.mult)
                # weff = wadd2 + tmp (all-bf16, SBUF only)
                nc.vector.tensor_add(weff[j][:], wadd2[j][:], tmp[:])

            # ---- Phase B helpers
            # m-outer matmul order: each psum's first write (re-inject or
            # first MM) sits one m-segment after the previous, so with a
            # 6-deep psum pool the flush of the previous chunk overlaps the
            # start of the next without stalling the PE.
            def mm_span(g, k0, k1, psums, reinj, stop_last):
                slabs = []
                k = k0
                while k < k1:
                    ke = min(k + XSLAB, k1)
                    xg = x_pool.tile([128, XSLAB * TG], bf16, tag="xg",
                                     name="xg")
                    nc.sync.dma_start(
                        out=xg[:, ds(0, (ke - k) * TG)],
                        in_=xt[ds(g * 128, 128), ds(k * TG, (ke - k) * TG)])
                    slabs.append((k, ke, xg))
                    k = ke
                for m in range(MPG):
                    if reinj is not None:
                        nc.tensor.matmul(psums[m][:], id_sb[:],
                                         reinj[m][:], start=True, stop=False)
                    for kb, ke, xg in slabs:
                        for s in range(ke - kb):
                            kk = kb + s
                            nc.tensor.matmul(
                                psums[m][:],
                                xg[:, ds(s * TG + m * 128, 128)],
                                weff[kk][:],
                                start=(reinj is None) and (kk == k0),
                                stop=stop_last and (kk == k1 - 1))

            partials = {}

            def emit_chunk(g, ci, k0, k1):
                psums = [mm_pool.tile([128, OSH], f32, tag="mmps",
                                      name="mmps") for _ in range(MPG)]
                reinj = [partials[(g, m)] for m in range(MPG)] if ci > 0 \
                    else None
                mm_span(g, k0, k1, psums, reinj, True)
                for m in range(MPG):
                    if ci == 0:
                        partials[(g, m)] = part_pool.tile(
                            [128, OSH], bf16, tag=f"part{g}_{m}",
                            name=f"part{g}_{m}")
                    nc.scalar.copy(partials[(g, m)][:], psums[m][:])

            def emit_b2(g):
                psums = [mm_pool.tile([128, OSH], f32, tag="mmps",
                                      name="mmps") for _ in range(MPG)]
                mm_span(g, KH, KT, psums, None, True)
                for m in range(MPG):
                    ot = o_pool.tile([128, OSH], f32, tag="ot", name="ot")
                    nc.vector.tensor_add(ot[:], psums[m][:],
                                         partials[(g, m)][:])
                    nc.scalar.dma_start(
                        out=out[ds(g * TG + m * 128, 128), :], in_=ot[:])

            def emit_b3(g):
                psums = [mm_pool.tile([128, OSH], f32, tag="mmps",
                                      name="mmps") for _ in range(MPG)]
                mm_span(g, 0, KT, psums, None, True)
                for m in range(MPG):
                    ot = o_pool.tile([128, OSH], f32, tag="ot", name="ot")
                    nc.scalar.copy(ot[:], psums[m][:])
                    nc.scalar.dma_start(
                        out=out[ds(g * TG + m * 128, 128), :], in_=ot[:])

            # B1 chunk table: fine chunks for the first groups so the PE has
            # work as soon as weff[0] lands; coarse (one flush) for the rest.
            bounds_for = {0: [0, 1, 2, 4, 8, 12, KH],
                          1: [0, 2, 4, 8, 12, KH],
                          2: [0, 4, 8, 12, KH]}
            by_k1 = {}
            for g in range(NSPLIT):
                bounds = bounds_for.get(g, [0, 8, KH])
                for ci in range(len(bounds) - 1):
                    by_k1.setdefault(bounds[ci + 1], []).append(
                        (g, ci, bounds[ci]))

            # Interleave phase A with the B1 chunks that each new weff tile
            # unlocks, so both the dequant chains and the PE stream flow in
            # weff-availability order.  Inputs lead the chain by one tile;
            # the LoRA products are spread over the first 7 iterations so
            # all of them clear the ACT stream before the heavy (PE-gated)
            # partial copies of the k1=8 wave.
            emit_inputs(0)
            ab_next = 0
            for j in range(KT):
                while ab_next < KT and ab_next < 5 * (j + 1):
                    emit_ab(ab_next)
                    ab_next += 1
                if j + 1 < KT:
                    emit_inputs(j + 1)
                emit_chain(j)
                for g, ci, k0 in by_k1.get(j + 1, []):
                    emit_chunk(g, ci, k0, j + 1)

            # Tail: second halves of split groups + the unsplit groups.
            b2s = list(range(NSPLIT))
            b3s = list(range(NSPLIT, NG))
            order = []
            while b2s or b3s:
                if b2s:
                    order.append(("b2", b2s.pop(0)))
                if b3s:
                    order.append(("b3", b3s.pop(0)))
            for kind, g in order:
                (emit_b2 if kind == "b2" else emit_b3)(g)

    nc.compile()
    _CACHE["nc"] = nc
    return nc


def _prepare_in_maps(x, q_idx, absmax, lora_A, lora_B):
    x = np.asarray(x, dtype=np.float32)
    q_idx = np.asarray(q_idx, dtype=np.int32)
    absmax = np.asarray(absmax, dtype=np.float32)
    lora_A = np.asarray(lora_A, dtype=np.float32)
    lora_B = np.asarray(lora_B, dtype=np.float32)

    bf = ml_dtypes.bfloat16
    # grouped x layout: [g, t, kt, p] -> [g, p, kt, t] -> row g*128+p
    x2 = np.ascontiguousarray(
        x.reshape(TOK, IN).reshape(NG, TG, KT, 128).transpose(0, 3, 2, 1)
    ).reshape(NG * 128, KT * TG).astype(bf)
    # u = fp16((2q-15)/15), transposed  [IN, OUT]
    u_full = ((2.0 * q_idx.T.astype(np.float32) - 15.0) / 15.0
              ).astype(np.float16)
    at = np.ascontiguousarray(lora_A.T).astype(bf)           # [R, IN]
    ident = np.eye(128, dtype=np.float32).astype(bf)
    wrm = np.zeros((128, OSH), dtype=bf)

    in_maps = []
    for cid in range(NCORES):
        sl = slice(cid * OSH, (cid + 1) * OSH)
        scale = np.repeat(np.ascontiguousarray(absmax[sl].T), QBLOCK,
                          axis=0).astype(bf)                 # [IN, OSH]
        in_maps.append({
            "xt": x2,
            "ident": ident,
            "wrm": wrm,
            "ut": np.ascontiguousarray(u_full[:, sl]),
            "scl": scale,
            "at": at,
            "bsh": np.ascontiguousarray(SCALING * lora_B[:, sl]).astype(bf),
        })
    return in_maps


def _gather(results):
    shards = [results[cid]["out"] for cid in range(NCORES)]
    full = np.concatenate(shards, axis=1)                    # [TOK, OUT]
    return full.reshape(B_, S_, OUT)


def kernel(x, q_idx, absmax, lora_A, lora_B):
    from concourse.bass_utils import run_bass_kernel_spmd

    nc = _build()
    in_maps = _prepare_in_maps(x, q_idx, absmax, lora_A, lora_B)
    res = run_bass_kernel_spmd(nc, in_maps, list(range(NCORES)))
    return _gather(res.results)
